# revision 7
# baseline (speedup 1.0000x reference)
"""Distributed kNN retrieval kernel for Trainium2 (8 NeuronCores).

Computes: ||x - y|| / 2 + mean(10 smallest ||data_i - x||)  over 2M rows.

Strategy (per the standard distributed-kNN recipe):
  - Shard `data` row-wise across 8 cores (250k rows each, padded to 251,904).
  - Each core's shard is laid out transposed on host: dataT [D=128, N_c] so the
    feature dim sits on SBUF partitions.  Then:
      ACT:  sq = Square(dataT + (-x))        (bias is per-partition = per-dim)
      PE :  psum[t, :] -= sum_d sq[d, :]     (stationary = -1 basis column,
                                              tile index t = output partition)
      ACT:  v = 4096 - d^2                   (PSUM -> SBUF evacuation)
      DVE:  max8 x2 + match_replace          -> top-16 candidates/partition
  - Host gathers 8 x [128,16] candidate values and reduces to the global
    top-10, then finishes the scalar math in numpy.

The kernel streams 1 MiB tiles; the whole thing is HBM-bandwidth bound
(~125 MB/core) with ACT/PE/DVE all comfortably under the DMA roofline.
"""

import numpy as np

import concourse.bacc as bacc
import concourse.mybir as mybir
from concourse.bass_utils import run_bass_kernel_spmd
from concourse.tile import TileContext

D = 128                 # feature dim
N_DATA = 2_000_000      # total database rows
NB_SOFTMIN = 10
MANIFOLD_SPEED = 2.0
N_CORES = 8

F = 2048                # rows per tile (free dim of one streamed tile)
TILES = 123             # tiles per core
N_C = F * TILES         # padded rows per core = 251,904
ROWS_PER_CORE = N_DATA // N_CORES  # 250,000
C_OFF = 4096.0          # v = C_OFF - d^2  (keeps values positive, low ulp)
PAD_VAL = 100.0         # pad-row fill -> d^2 ~ 1.3e6, never in top-k
NEG_BIG = -3.0e38       # match_replace fill

_CACHE = {}


def _build_nc(reps=1):
    nc = bacc.Bacc("TRN2")
    data_t = nc.dram_tensor("data_t", [D, N_C], mybir.dt.float32,
                            kind="ExternalInput")
    neg_x = nc.dram_tensor("neg_x", [D, 1], mybir.dt.float32,
                           kind="ExternalInput")
    bconst = nc.dram_tensor("bconst", [D, 256], mybir.dt.float32,
                            kind="ExternalInput")
    cand = nc.dram_tensor("cand", [D, 16], mybir.dt.float32,
                          kind="ExternalOutput")

    FT = mybir.dt.float32
    AF = mybir.ActivationFunctionType

    with TileContext(nc) as tc:
        with (
            tc.tile_pool(name="consts", bufs=1) as consts,
            tc.tile_pool(name="data", bufs=3) as data_pool,
            tc.tile_pool(name="sq", bufs=3) as sq_pool,
            tc.tile_pool(name="store", bufs=1) as store,
            tc.tile_pool(name="psum", bufs=1, space="PSUM") as psum_pool,
        ):
            mx_sb = consts.tile([D, 1], FT)
            nc.sync.dma_start(out=mx_sb[:, :], in_=neg_x[:, :])
            b_sb = consts.tile([D, 256], FT)
            nc.sync.dma_start(out=b_sb[:, :], in_=bconst[:, :])

            # 4 PSUM banks worth of accumulated (negated) distances.
            pacc = psum_pool.tile([D, 4 * 512], FT)

            import contextlib
            rep_loop = (tc.For_i(0, reps, 1) if reps > 1
                        else contextlib.nullcontext())
            with rep_loop:
                _body(nc, tc, data_t, cand, mx_sb, b_sb, pacc,
                      data_pool, sq_pool, store, AF, FT)

    nc.compile()
    return nc


def _body(nc, tc, data_t, cand, mx_sb, b_sb, pacc, data_pool, sq_pool, store,
          AF, FT):
    if True:
        if True:
            for t in range(TILES):
                dt_tile = data_pool.tile([D, F], FT)
                nc.sync.dma_start(out=dt_tile[:, :],
                                  in_=data_t[:, t * F:(t + 1) * F])
                sq = sq_pool.tile([D, F], FT)
                nc.scalar.activation(out=sq[:, :], in_=dt_tile[:, :],
                                     func=AF.Square, bias=mx_sb[:, :],
                                     scale=1.0)
                for j in range(4):
                    nc.tensor.matmul(
                        pacc[:, j * 512:(j + 1) * 512],
                        b_sb[:, 128 - t:256 - t],
                        sq[:, j * 512:(j + 1) * 512],
                        start=(t == 0),
                        stop=(t == TILES - 1),
                    )

            # v = C_OFF - d^2   (rows of pacc hold -d^2 per 512-row chunk)
            v = store.tile([D, 4 * 512], FT)
            for j in range(4):
                nc.scalar.activation(out=v[:, j * 512:(j + 1) * 512],
                                     in_=pacc[:, j * 512:(j + 1) * 512],
                                     func=AF.Copy, bias=C_OFF, scale=1.0)

            # Top-16 values per partition: max8, zap them, max8 again.
            t8a = store.tile([D, 8], FT)
            nc.vector.max(out=t8a[:, :], in_=v[:, :])
            vrep = store.tile([D, 4 * 512], FT)
            nc.vector.match_replace(out=vrep[:, :], in_to_replace=t8a[:, :],
                                    in_values=v[:, :], imm_value=NEG_BIG)
            t8b = store.tile([D, 8], FT)
            nc.vector.max(out=t8b[:, :], in_=vrep[:, :])

            nc.sync.dma_start(out=cand[:, 0:8], in_=t8a[:, :])
            nc.sync.dma_start(out=cand[:, 8:16], in_=t8b[:, :])


def _get_nc():
    if "nc" not in _CACHE:
        _CACHE["nc"] = _build_nc()
    return _CACHE["nc"]


def _make_in_maps(x, data):
    neg_x = np.ascontiguousarray((-x).reshape(D, 1), dtype=np.float32)
    bconst = np.zeros((D, 256), dtype=np.float32)
    bconst[:, 128] = -1.0
    in_maps = []
    for c in range(N_CORES):
        lo = c * ROWS_PER_CORE
        hi = lo + ROWS_PER_CORE
        shard_t = np.full((D, N_C), PAD_VAL, dtype=np.float32)
        shard_t[:, :ROWS_PER_CORE] = data[lo:hi].T
        in_maps.append({
            "data_t": np.ascontiguousarray(shard_t),
            "neg_x": neg_x,
            "bconst": bconst,
        })
    return in_maps


def _postprocess(x, y, results):
    cands = np.concatenate(
        [np.asarray(r["cand"], dtype=np.float32).reshape(-1) for r in results]
    )
    d2 = C_OFF - cands
    # Untouched PSUM rows (tile partitions 123-127) evacuate as exactly
    # C_OFF -> d2 == 0.  Real distances are strictly positive; drop them.
    d2 = d2[d2 > 1e-6]
    d2.sort()
    closest = np.sqrt(d2[:NB_SOFTMIN].astype(np.float32))
    xy = np.float32(np.linalg.norm((x - y).astype(np.float32)))
    return np.float32(xy / np.float32(MANIFOLD_SPEED)
                      + closest.mean(dtype=np.float32))


def kernel(x, y, data, _trace=False):
    x = np.asarray(x, dtype=np.float32)
    y = np.asarray(y, dtype=np.float32)
    data = np.asarray(data, dtype=np.float32)
    nc = _get_nc()
    in_maps = _make_in_maps(x, data)
    res = run_bass_kernel_spmd(nc, in_maps, core_ids=list(range(N_CORES)),
                               trace=_trace)
    out = _postprocess(x, y, res.results)
    if _trace:
        return out, res
    return out
